# revision 5
# baseline (speedup 1.0000x reference)
"""GAT layer (gnn_message_passing) Trainium2 Bass kernel, 8-core data-parallel.

Strategy:
  - Shard the fused B*T=1536 graph-replica axis: 192 graphs per core.
  - Per core, device pipeline:
      phase0: assemble lhsT W' = [W^T | W^T att_src | W^T att_dst]  (on device)
      phase1: h'[c',(n,g)] = W'^T @ xT   (xT host-marshalled to [F, n, g])
              staged to DRAM hstage [66, n, g]
      phase2: edge scalars: a_s/a_d gathered per edge via one-hot matmuls,
              leaky-relu, exp, segment-sum denominator via one-hot scatter
              matmul, alpha = ex * (1/den)[dst]   -- all at [slot, g] size.
      phase3: read back h as h_C [n_chunk, (c, g)] (affine DMA from hstage),
              main loop: per 512-wide free window (8 c x 64 g):
                gather-MM (one-hot) -> PSUM, DVE multiply by alpha
                (broadcast over c), scatter-MM (one-hot) accumulating into
                per-dst-chunk PSUM initialized with bias; Gelu fused into the
                PSUM->SBUF copy which also transposes free (c,g)->(g,c).
      out DMA to [g, n, c] which concatenated over cores is exactly the
      reference's raw reshape(B, N, T, H).

Edge structure (one-hot matrices, block schedule) is baked at build time from
the runtime edge_index values; heavy numeric work all happens on device.
"""

import os
import sys

try:
    import concourse  # noqa: F401
except ImportError:
    sys.path.insert(0, "/opt/trn_rl_repo")

import numpy as np

import concourse.bacc as bacc
import concourse.bass as bass  # noqa: F401
import concourse.mybir as mybir
import concourse.tile as tile
from concourse.bass_utils import run_bass_kernel_spmd
from concourse.masks import make_identity

F32 = mybir.dt.float32

B, N, T, F_IN, H, E = 64, 325, 24, 64, 64, 2600
NEG_SLOPE = 0.2
NCORES = 8
GT = B * T          # 1536 total graphs
G = GT // NCORES    # 192 graphs per core
NG = N * G          # 62400 free elems in stage layouts
C2 = H + 2          # 66 = h channels + a_s + a_d rows
CH = [0, 128, 256, N]          # node chunk boundaries
CSZ = [128, 128, N - 256]      # 128,128,69
KB = (E + 127) // 128          # 21 slot blocks
GTH = 64                       # g per "third" (192 = 3*64)
FS = 512                       # free window (8 c x 64 g)
CPW = FS // GTH                # 8 c per window
NSUP = H // CPW                # 12 windows per third... (64/8=8) per third


def _chunk_of(v):
    return 0 if v < 128 else (1 if v < 256 else 2)


def _build_schedule(src, dst):
    """Sort edges by (dst_chunk, src_chunk); build one-hot blocks + schedule."""
    src = np.asarray(src, dtype=np.int64)
    dst = np.asarray(dst, dtype=np.int64)
    scc = np.minimum(src // 128, 2)
    dcc = np.minimum(dst // 128, 2)
    perm = np.lexsort((scc, dcc))  # primary dcc, secondary scc
    ss, dd = src[perm], dst[perm]

    S_blocks, D_blocks, D2_blocks = [], [], []
    gather, scatter = [], []  # per kb: list of (sc, S_idx) / (dc, D_idx)
    for kb in range(KB):
        lo, hi = kb * 128, min((kb + 1) * 128, E)
        g_pieces, s_pieces = [], []
        by_sc, by_dc = {}, {}
        for slot in range(lo, hi):
            by_sc.setdefault(_chunk_of(ss[slot]), []).append(slot)
            by_dc.setdefault(_chunk_of(dd[slot]), []).append(slot)
        for sc in sorted(by_sc):
            M = np.zeros((128, 128), dtype=np.float32)
            for slot in by_sc[sc]:
                M[ss[slot] - CH[sc], slot - lo] = 1.0
            g_pieces.append((sc, len(S_blocks)))
            S_blocks.append(M)
        for dc in sorted(by_dc):
            Dm = np.zeros((128, 128), dtype=np.float32)
            D2m = np.zeros((128, 128), dtype=np.float32)
            for slot in by_dc[dc]:
                Dm[slot - lo, dd[slot] - CH[dc]] = 1.0
                D2m[dd[slot] - CH[dc], slot - lo] = 1.0
            s_pieces.append((dc, len(D_blocks)))
            D_blocks.append(Dm)
            D2_blocks.append(D2m)
        gather.append(g_pieces)
        scatter.append(s_pieces)

    # first/last kb contributing to each dst chunk (for start/stop flags)
    first_kb = {}
    last_kb = {}
    for kb in range(KB):
        for dc, _ in scatter[kb]:
            first_kb.setdefault(dc, kb)
            last_kb[dc] = kb
    return {
        "S": np.concatenate(S_blocks, axis=1),
        "D": np.concatenate(D_blocks, axis=1),
        "D2": np.concatenate(D2_blocks, axis=1),
        "gather": gather,
        "scatter": scatter,
        "first_kb": first_kb,
        "last_kb": last_kb,
    }


def _build_program(sched):
    nS = sched["S"].shape[1] // 128
    nD = sched["D"].shape[1] // 128

    nc = bacc.Bacc("TRN2", target_bir_lowering=False, debug=False,
                   enable_asserts=False)
    xT_d = nc.dram_tensor("xT", [F_IN, NG], F32, kind="ExternalInput")
    S_d = nc.dram_tensor("S_pack", [128, nS * 128], F32, kind="ExternalInput")
    D_d = nc.dram_tensor("D_pack", [128, nD * 128], F32, kind="ExternalInput")
    D2_d = nc.dram_tensor("D2_pack", [128, nD * 128], F32, kind="ExternalInput")
    W_d = nc.dram_tensor("W", [H, F_IN], F32, kind="ExternalInput")
    asrc_d = nc.dram_tensor("att_src", [H, 1], F32, kind="ExternalInput")
    adst_d = nc.dram_tensor("att_dst", [H, 1], F32, kind="ExternalInput")
    brow_d = nc.dram_tensor("biasrow", [1, H * GTH], F32, kind="ExternalInput")
    ones_d = nc.dram_tensor("ones1", [1, 128], F32, kind="ExternalInput")
    hstage_d = nc.dram_tensor("hstage", [C2, NG], F32)
    out_d = nc.dram_tensor("out", [G, N, H], F32, kind="ExternalOutput")

    hstage_r = hstage_d.ap().rearrange("c (n g) -> c n g", g=G)
    out_r = out_d.ap()

    gather, scatter = sched["gather"], sched["scatter"]
    first_kb, last_kb = sched["first_kb"], sched["last_kb"]

    with tile.TileContext(nc) as tc:
        # ---------------- constants ----------------
        with tc.tile_pool(name="const", bufs=1) as cpool:
            S_sb = cpool.tile([128, nS * 128], F32)
            nc.sync.dma_start(S_sb[:], S_d.ap())
            D_sb = cpool.tile([128, nD * 128], F32)
            nc.sync.dma_start(D_sb[:], D_d.ap())
            D2_sb = cpool.tile([128, nD * 128], F32)
            nc.sync.dma_start(D2_sb[:], D2_d.ap())
            brow_sb = cpool.tile([1, H * GTH], F32)
            nc.sync.dma_start(brow_sb[:], brow_d.ap())
            ones_sb = cpool.tile([1, 128], F32)
            nc.sync.dma_start(ones_sb[:], ones_d.ap())
            alpha_sb = cpool.tile([128, KB * G], F32)   # per-edge final weights
            lhsT_W = cpool.tile([F_IN, C2], F32)

            # ---------------- phase 0: W' assembly ----------------
            with tc.tile_pool(name="ph0", bufs=1) as p0, \
                 tc.tile_pool(name="ph0ps", bufs=1, space="PSUM") as p0ps:
                W_sb = p0.tile([H, F_IN], F32)
                nc.sync.dma_start(W_sb[:], W_d.ap())
                ident = p0.tile([H, H], F32)
                make_identity(nc, ident[:])
                wt_ps = p0ps.tile([F_IN, H], F32)
                nc.tensor.transpose(wt_ps[:], W_sb[:], ident[:])
                nc.vector.tensor_copy(lhsT_W[:, 0:H], wt_ps[:])
                asrc_sb = p0.tile([H, 1], F32)
                nc.sync.dma_start(asrc_sb[:], asrc_d.ap())
                adst_sb = p0.tile([H, 1], F32)
                nc.sync.dma_start(adst_sb[:], adst_d.ap())
                ws_ps = p0ps.tile([F_IN, 2], F32)
                nc.tensor.matmul(ws_ps[:, 0:1], W_sb[:], asrc_sb[:],
                                 start=True, stop=True)
                nc.tensor.matmul(ws_ps[:, 1:2], W_sb[:], adst_sb[:],
                                 start=True, stop=True)
                nc.vector.tensor_copy(lhsT_W[:, H:C2], ws_ps[:])

            # ---------------- phase 1: h' = W'^T x, stage to DRAM ----------
            CH1 = 2048
            with tc.tile_pool(name="ph1x", bufs=3) as p1x, \
                 tc.tile_pool(name="ph1h", bufs=3) as p1h, \
                 tc.tile_pool(name="ph1ps", bufs=2, space="PSUM") as p1ps:
                nch = (NG + CH1 - 1) // CH1
                for i in range(nch):
                    lo = i * CH1
                    w = min(CH1, NG - lo)
                    xt = p1x.tile([F_IN, CH1], F32, tag="xt")
                    nc.sync.dma_start(xt[:, 0:w], xT_d.ap()[:, lo:lo + w])
                    hp = p1ps.tile([C2, CH1], F32, tag="hp")
                    for j in range(0, w, 512):
                        wj = min(512, w - j)
                        nc.tensor.matmul(hp[:, j:j + wj], lhsT_W[:],
                                         xt[:, j:j + wj], start=True, stop=True)
                    hsb = p1h.tile([C2, CH1], F32, tag="hsb")
                    if i % 2 == 0:
                        nc.scalar.activation(hsb[:, 0:w], hp[:, 0:w],
                                             mybir.ActivationFunctionType.Copy)
                    else:
                        nc.vector.tensor_copy(hsb[:, 0:w], hp[:, 0:w])
                    nc.sync.dma_start(hstage_d.ap()[:, lo:lo + w], hsb[:, 0:w])

            # ---------------- phase 2: edge scalars -> alpha ---------------
            with tc.tile_pool(name="ph2", bufs=4) as p2, \
                 tc.tile_pool(name="ph2ex", bufs=1) as p2ex, \
                 tc.tile_pool(name="ph2a", bufs=1) as p2a, \
                 tc.tile_pool(name="ph2ps", bufs=2, space="PSUM") as p2ps, \
                 tc.tile_pool(name="ph2psr", bufs=1, space="PSUM") as p2psr, \
                 tc.tile_pool(name="ph2psd", bufs=1, space="PSUM") as p2psd:
                asel = []
                for sc in range(3):
                    t_as = p2a.tile([CSZ[sc], G], F32, tag=f"as{sc}")
                    nc.sync.dma_start(
                        t_as[:], hstage_r[H, CH[sc]:CH[sc + 1], :])
                    t_ad = p2a.tile([CSZ[sc], G], F32, tag=f"ad{sc}")
                    nc.sync.dma_start(
                        t_ad[:], hstage_r[H + 1, CH[sc]:CH[sc + 1], :])
                    asel.append((t_as, t_ad))

                ex_sb = p2ex.tile([128, KB * G], F32)
                den_ps = [p2psd.tile([128, G], F32, tag=f"den{dc}", name=f"den{dc}")
                          for dc in range(3)]
                for kb in range(KB):
                    # e[slot,g] = a_s[src]+a_d[dst]: both gathers accumulate
                    # into ONE PSUM tile (S one-hots on a_s, D2 on a_d).
                    aps = p2ps.tile([128, G], F32, tag="aps")
                    npc = len(gather[kb]) + len(scatter[kb])
                    i = 0
                    for sc, sidx in gather[kb]:
                        nc.tensor.matmul(
                            aps[:], S_sb[0:CSZ[sc], sidx * 128:(sidx + 1) * 128],
                            asel[sc][0][:], start=(i == 0), stop=(i == npc - 1))
                        i += 1
                    for dc, didx in scatter[kb]:
                        nc.tensor.matmul(
                            aps[:],
                            D2_sb[0:CSZ[dc], didx * 128:(didx + 1) * 128],
                            asel[dc][1][:], start=(i == 0), stop=(i == npc - 1))
                        i += 1
                    e2 = p2.tile([128, G], F32, tag="e2")
                    nc.vector.tensor_scalar_mul(e2[:], aps[:], NEG_SLOPE)
                    elr = p2.tile([128, G], F32, tag="elr")
                    nc.vector.tensor_tensor(elr[:], aps[:], e2[:],
                                            mybir.AluOpType.max)
                    nc.scalar.activation(ex_sb[:, kb * G:(kb + 1) * G], elr[:],
                                         mybir.ActivationFunctionType.Exp)
                    for dc, didx in scatter[kb]:
                        nc.tensor.matmul(
                            den_ps[dc][0:CSZ[dc], :],
                            D_sb[:, didx * 128:didx * 128 + CSZ[dc]],
                            ex_sb[:, kb * G:(kb + 1) * G],
                            start=(first_kb[dc] == kb),
                            stop=(last_kb[dc] == kb))
                rden = []
                for dc in range(3):
                    dsb = p2.tile([CSZ[dc], G], F32, tag=f"dsb{dc}")
                    nc.vector.tensor_scalar_max(dsb[:], den_ps[dc][0:CSZ[dc], :],
                                                1e-30)
                    rd = p2a.tile([CSZ[dc], G], F32, tag=f"rden{dc}")
                    nc.vector.reciprocal(rd[:], dsb[:])
                    rden.append(rd)
                for kb in range(KB):
                    rps = p2psr.tile([128, G], F32, tag="rps")
                    for i, (dc, didx) in enumerate(scatter[kb]):
                        fl = (i == 0, i == len(scatter[kb]) - 1)
                        nc.tensor.matmul(
                            rps[:],
                            D2_sb[0:CSZ[dc], didx * 128:(didx + 1) * 128],
                            rden[dc][:], start=fl[0], stop=fl[1])
                    nc.vector.tensor_tensor(
                        alpha_sb[:, kb * G:(kb + 1) * G], rps[:],
                        ex_sb[:, kb * G:(kb + 1) * G], mybir.AluOpType.mult)

            # ---------------- phase 3: main gather/scatter loop ------------
            with tc.tile_pool(name="hc", bufs=1) as phc, \
                 tc.tile_pool(name="osb", bufs=1) as posb, \
                 tc.tile_pool(name="mm", bufs=3) as pm, \
                 tc.tile_pool(name="mps", bufs=2, space="PSUM") as pmps, \
                 tc.tile_pool(name="bps", bufs=2, space="PSUM") as pbps:
                for third in range(3):
                    g0 = third * GTH
                    hC = []
                    for sc in range(3):
                        t = phc.tile([CSZ[sc], H * GTH], F32, tag=f"hc{sc}")
                        nc.sync.dma_start(
                            t[:].rearrange("p (c g) -> p c g", c=H),
                            hstage_r[0:H, CH[sc]:CH[sc + 1],
                                     g0:g0 + GTH].transpose([1, 0, 2]))
                        hC.append(t)
                    outsb = [posb.tile([CSZ[dc], GTH * H], F32, tag=f"o{dc}", name=f"o{third}_{dc}")
                             for dc in range(3)]
                    for sup in range(NSUP):
                        w = sup * FS
                        c0 = sup * CPW
                        psB = [pbps.tile([128, FS], F32, tag=f"psB{dc}", name=f"psB{third}_{sup}_{dc}")
                               for dc in range(3)]
                        for dc in range(3):
                            nc.tensor.matmul(
                                psB[dc][0:CSZ[dc], :], ones_sb[0:1, 0:CSZ[dc]],
                                brow_sb[:, w:w + FS], start=True, stop=False)
                        for kb in range(KB):
                            psA = pmps.tile([128, FS], F32, tag="psA")
                            for i, (sc, sidx) in enumerate(gather[kb]):
                                fl = (i == 0, i == len(gather[kb]) - 1)
                                nc.tensor.matmul(
                                    psA[:],
                                    S_sb[0:CSZ[sc],
                                         sidx * 128:(sidx + 1) * 128],
                                    hC[sc][:, w:w + FS],
                                    start=fl[0], stop=fl[1])
                            m = pm.tile([128, FS], F32, tag="m")
                            acol = kb * G + g0
                            nc.vector.tensor_tensor(
                                m[:].rearrange("p (c g) -> p c g", c=CPW),
                                psA[:].rearrange("p (c g) -> p c g", c=CPW),
                                alpha_sb[:, None, acol:acol + GTH]
                                .to_broadcast([128, CPW, GTH]),
                                mybir.AluOpType.mult)
                            for dc, didx in scatter[kb]:
                                nc.tensor.matmul(
                                    psB[dc][0:CSZ[dc], :],
                                    D_sb[:, didx * 128:didx * 128 + CSZ[dc]],
                                    m[:], start=False,
                                    stop=(last_kb[dc] == kb))
                        for dc in range(3):
                            # gelu + transpose free (c,g)->(g,c)
                            nc.scalar.activation(
                                outsb[dc][:].rearrange(
                                    "p (g c) -> p c g", c=H)[:, c0:c0 + CPW, :],
                                psB[dc][0:CSZ[dc], :].rearrange(
                                    "p (c g) -> p c g", c=CPW),
                                mybir.ActivationFunctionType.Gelu)
                    for dc in range(3):
                        nc.sync.dma_start(
                            out_r[g0:g0 + GTH, CH[dc]:CH[dc + 1], :]
                            .transpose([1, 0, 2]),
                            outsb[dc][:].rearrange("p (g c) -> p g c", g=GTH))

    nc.finalize()
    return nc


_CACHE = {}


def kernel(x, edge_index, W, att_src, att_dst, bias):
    x = np.asarray(x, dtype=np.float32)
    edge_index = np.asarray(edge_index)
    W = np.asarray(W, dtype=np.float32)
    att_src = np.asarray(att_src, dtype=np.float32)
    att_dst = np.asarray(att_dst, dtype=np.float32)
    bias = np.asarray(bias, dtype=np.float32)

    src, dst = edge_index[0], edge_index[1]
    key = (src.tobytes(), dst.tobytes())
    if key not in _CACHE:
        sched = _build_schedule(src, dst)
        _CACHE[key] = (_build_program(sched), sched)
    nc, sched = _CACHE[key]

    # host marshalling: fold replicas, shard, transpose to [F, n, g]
    xg = np.ascontiguousarray(x.transpose(0, 2, 1, 3)).reshape(GT, N, F_IN)
    biasrow = np.ascontiguousarray(
        np.repeat(bias, GTH)[None, :])          # [1, 64*64], c-major g-minor
    common = {
        "S_pack": np.ascontiguousarray(sched["S"]),
        "D_pack": np.ascontiguousarray(sched["D"]),
        "D2_pack": np.ascontiguousarray(sched["D2"]),
        "W": W,
        "att_src": np.ascontiguousarray(att_src.reshape(H, 1)),
        "att_dst": np.ascontiguousarray(att_dst.reshape(H, 1)),
        "biasrow": biasrow,
        "ones1": np.ones((1, 128), dtype=np.float32),
    }
    in_maps = []
    for c in range(NCORES):
        xc = xg[c * G:(c + 1) * G]                       # [G, N, F]
        xT = np.ascontiguousarray(xc.transpose(2, 1, 0))  # [F, N, G]
        in_maps.append({**common, "xT": xT.reshape(F_IN, NG)})

    res = run_bass_kernel_spmd(nc, in_maps, core_ids=list(range(NCORES)))
    full = np.concatenate([res.results[c]["out"] for c in range(NCORES)],
                          axis=0)                        # [GT, N, H]
    return full.reshape(B, N, T, H)
